# revision 1
# baseline (speedup 1.0000x reference)
"""EGAT (edge-featured GAT) kernel for 8 Trainium2 NeuronCores.

Edge-parallel sharding: edges are sorted by dst and split into 8 contiguous
shards at dst boundaries, so each core owns a disjoint dst range end-to-end
(softmax stats + aggregation are fully local -> no collectives).

Per core the edges are packed into W windows of 2048 edge slots (16 tiles of
128).  A window spans at most 128 distinct dst rows.  The host gathers the
src/dst feature rows per edge slot (edge-aligned layout, transposed so the
feature dim sits on partitions); the device projects them tile-by-tile on
the TensorEngine, accumulating f_ni + f_nj + r*wsum + b_e in one PSUM tile
(r*wsum and b_e ride along via host-crafted virtual feature rows x_row and
y_row with x_row @ W_nj = colsum(W_fij), y_row @ W_nj = b_e).  Attention
weights (leaky_relu -> attn dot -> exp, no max-subtraction needed: logits
are O(1)) are computed on DVE/ACT, and a one-hot scatter matmul accumulates
both the weighted messages and the softmax denominators into one PSUM tile
per window.  The epilogue normalizes, averages heads, adds mean(b_ns),
applies relu, and DMAs the window's 128 output rows.
"""

import sys

sys.path.insert(0, "/opt/trn_rl_repo")

import numpy as np
import ml_dtypes

BF16 = ml_dtypes.bfloat16

# ---- problem constants (hardcoded per the task contract) -------------------
N_SRC = 50000
N_DST = 50000
E = 800000
IN_NODE = 128
IN_EDGE = 16
OUT_NODE = 64
OUT_EDGE = 16
H = 4
SLOPE = 0.01

N_CORES = 8


def default_cfg():
    return dict(
        n_dst=N_DST,
        t_half=8,             # tiles per half-window (8 -> 1024 slots/half)
        span=128,             # max dst rows per window
    )


# ===========================================================================
# Host-side packing
# ===========================================================================

def prep(nfeats, dst_feats, reward, src, dst, W_ns, b_ns, W_ni, W_nj, W_fij,
         attn, b_e, cfg=None):
    """Sort/shard/pack everything. Returns (meta, in_maps)."""
    cfg = cfg or default_cfg()
    n_dst = cfg["n_dst"]
    t_half = cfg["t_half"]
    span = cfg["span"]
    slots = 2 * t_half * 128    # slots per window
    t_w = 2 * t_half

    e_tot = src.shape[0]

    nfeats = np.asarray(nfeats, np.float32)
    dst_feats = np.asarray(dst_feats, np.float32)
    reward = np.asarray(reward, np.float32)
    src = np.asarray(src, np.int64)
    dst = np.asarray(dst, np.int64)
    W_ns = np.asarray(W_ns, np.float32)
    b_ns = np.asarray(b_ns, np.float32)
    W_ni = np.asarray(W_ni, np.float32)
    W_nj = np.asarray(W_nj, np.float32)
    W_fij = np.asarray(W_fij, np.float32)
    attn = np.asarray(attn, np.float32)
    b_e = np.asarray(b_e, np.float32)

    # ---- sort by dst and shard at dst boundaries --------------------------
    order = np.argsort(dst, kind="stable")
    d_s = dst[order]
    s_s = src[order]
    r_s = reward[order]

    cut = [0]
    for c in range(1, N_CORES):
        t = (e_tot * c) // N_CORES
        while t < e_tot and t > 0 and d_s[t] == d_s[t - 1]:
            t += 1
        cut.append(t)
    cut.append(e_tot)

    # ---- greedy window packing per core -----------------------------------
    per_core = []
    for c in range(N_CORES):
        e0, e1 = cut[c], cut[c + 1]
        d = d_s[e0:e1]
        wins = []  # (base, n_edges) over local positions (contiguous runs)
        if e1 > e0:
            uniq, starts = np.unique(d, return_index=True)
            ends = np.append(starts[1:], len(d))
            base = None
            w_start = 0
            w_count = 0
            for gi in range(len(uniq)):
                dd = int(uniq[gi])
                glen = int(ends[gi] - starts[gi])
                if (base is None or dd - base > span - 1
                        or w_count + glen > slots):
                    if base is not None:
                        wins.append((base, w_start, w_count))
                    base = dd
                    w_start = int(starts[gi])
                    w_count = 0
                w_count += glen
            wins.append((base, w_start, w_count))
        per_core.append((e0, e1, wins))

    W = max(1, max(len(pc[2]) for pc in per_core))

    # virtual feature rows: x_row @ W_nj == colsum(W_fij); y_row @ W_nj == b_e
    wsum = W_fij.sum(axis=0)
    x_row = np.linalg.lstsq(W_nj.T.astype(np.float64), wsum.astype(np.float64),
                            rcond=None)[0].astype(np.float32)
    y_row = np.linalg.lstsq(W_nj.T.astype(np.float64), b_e.astype(np.float64),
                            rcond=None)[0].astype(np.float32)

    mf_all = []     # [128, W, t_w, 128] bf16 one-hot per slot
    zfe_all = []    # [128, W*2*slots] bf16 (src feats | dst feats per window)
    asm = []        # per core (slot_rows, global_rows)

    for c in range(N_CORES):
        e0, e1, wins = per_core[c]
        d = d_s[e0:e1]
        s = s_s[e0:e1]
        r = r_s[e0:e1]

        drel = np.full((W, slots), -1.0, np.float32)
        nfe = np.zeros((W * slots, IN_NODE), np.float32)
        dfe = np.zeros((W * slots, IN_NODE), np.float32)
        rows_slot = []
        rows_glob = []
        for w, (base, ws, wc) in enumerate(wins):
            sl = slice(ws, ws + wc)
            drel[w, :wc] = (d[sl] - base).astype(np.float32)
            nfe[w * slots:w * slots + wc] = nfeats[s[sl]]
            dfe[w * slots:w * slots + wc] = (dst_feats[d[sl]]
                                             + r[sl, None] * x_row[None, :]
                                             + y_row[None, :])
            uds = np.unique(d[sl])
            rows_slot.append(w * 128 + (uds - base))
            rows_glob.append(uds)

        # one-hot per slot, layout [128 p, W, t, 128 dcol]
        ohm = (drel.reshape(W, t_w, 128)[:, :, :, None]
               == np.arange(128, dtype=np.float32)).astype(BF16)
        ohm = np.ascontiguousarray(ohm.transpose(2, 0, 1, 3))

        zfe = np.empty((IN_NODE, W, 2 * slots), np.float32)
        zfe[:, :, :slots] = nfe.T.reshape(IN_NODE, W, slots)
        zfe[:, :, slots:] = dfe.T.reshape(IN_NODE, W, slots)
        mf_all.append(ohm)
        zfe_all.append(np.ascontiguousarray(
            zfe.reshape(IN_NODE, W * 2 * slots).astype(BF16)))
        asm.append((np.concatenate(rows_slot) if rows_slot else
                    np.zeros(0, np.int64),
                    np.concatenate(rows_glob) if rows_glob else
                    np.zeros(0, np.int64)))

    # ---- shared constants -------------------------------------------------
    wcat = np.concatenate([W_ni, W_ns * 0.25], axis=1).astype(BF16)  # [128,320]
    wnj = W_nj.astype(BF16)                                          # [128,64]
    attn_rep = np.broadcast_to(attn.reshape(-1).astype(np.float32),
                               (128, H * OUT_EDGE)).astype(BF16).copy()
    bmean = np.broadcast_to(b_ns.reshape(H, OUT_NODE).mean(axis=0),
                            (128, OUT_NODE)).astype(np.float32).copy()

    in_maps = []
    for c in range(N_CORES):
        in_maps.append(dict(
            zfe=zfe_all[c], ohm=mf_all[c],
            wcat=wcat, wnj=wnj, attn_rep=attn_rep, bmean=bmean,
        ))

    meta = dict(W=W, asm=asm, cfg=cfg)
    return meta, in_maps


# ===========================================================================
# Device program
# ===========================================================================

def build_program(W, cfg):
    import concourse.bacc as bacc
    import concourse.tile as tile
    import concourse.mybir as mybir
    from contextlib import ExitStack

    dt = mybir.dt
    AF = mybir.ActivationFunctionType
    OP = mybir.AluOpType

    t_half = cfg["t_half"]
    t_w = 2 * t_half
    slots = t_w * 128
    FE = H * OUT_EDGE          # 64
    NPAY = H * OUT_NODE        # 256
    NPROJ = FE + NPAY          # 320

    nc = bacc.Bacc(None, target_bir_lowering=False)

    ZFE = nc.declare_dram_parameter("zfe", [IN_NODE, W * 2 * slots],
                                    dt.bfloat16, isOutput=False)
    OHM = nc.declare_dram_parameter("ohm", [128, W, 2 * t_half, 128],
                                    dt.bfloat16, isOutput=False)
    WCAT = nc.declare_dram_parameter("wcat", [IN_NODE, NPROJ], dt.bfloat16,
                                     isOutput=False)
    WNJ = nc.declare_dram_parameter("wnj", [IN_NODE, FE], dt.bfloat16,
                                    isOutput=False)
    ATTN = nc.declare_dram_parameter("attn_rep", [128, FE], dt.bfloat16,
                                     isOutput=False)
    BMEAN = nc.declare_dram_parameter("bmean", [128, OUT_NODE], dt.float32,
                                      isOutput=False)
    OUT = nc.declare_dram_parameter("out", [W * 128, OUT_NODE], dt.float32,
                                    isOutput=True)

    with tile.TileContext(nc) as tc, ExitStack() as ctx:
        cpool = ctx.enter_context(tc.tile_pool(name="consts", bufs=1))
        wcat_s = cpool.tile([128, NPROJ], dt.bfloat16)
        nc.sync.dma_start(wcat_s[:], WCAT[:])
        wnj_s = cpool.tile([128, FE], dt.bfloat16)
        nc.sync.dma_start(wnj_s[:], WNJ[:])
        out_acc = cpool.tile([128, W, OUT_NODE], dt.float32)
        attn_s = cpool.tile([128, FE], dt.bfloat16)
        nc.sync.dma_start(attn_s[:], ATTN[:])
        bmean_s = cpool.tile([128, OUT_NODE], dt.float32)
        nc.sync.dma_start(bmean_s[:], BMEAN[:])

        with tc.tile_pool(name="feat", bufs=3) as fpool, \
             tc.tile_pool(name="meta", bufs=3) as mpool, \
             tc.tile_pool(name="work", bufs=3) as wpool, \
             tc.tile_pool(name="stgp", bufs=3) as ppool, \
             tc.tile_pool(name="rhsp", bufs=3) as rpool, \
             tc.tile_pool(name="ep", bufs=2) as epool, \
             tc.tile_pool(name="psPr", bufs=2, space="PSUM") as psPr, \
             tc.tile_pool(name="psP", bufs=2, space="PSUM") as psP:
            for w in range(W):
                zfe = fpool.tile([128, 2 * slots], dt.bfloat16, tag="zfe")
                eng = nc.sync if w % 2 == 0 else nc.gpsimd
                eng.dma_start(zfe[:],
                              ZFE[:, w * 2 * slots:(w + 1) * 2 * slots])
                nfe = zfe[:, 0:slots]
                dfe = zfe[:, slots:2 * slots]
                ohw = mpool.tile([128, 2 * t_half, 128], dt.bfloat16, tag="ohw")
                (nc.gpsimd if w % 2 == 0 else nc.sync).dma_start(
                    ohw[:], OHM[:, w, :, :])

                P = psP.tile([128, NPAY + H], dt.float32, tag="P")
                rhs = rpool.tile([128, t_w, NPAY + H], dt.bfloat16, tag="rhs")
                stg = ppool.tile([128, t_w, NPROJ], dt.bfloat16, tag="stg")

                for hf in range(2):
                    ts0 = hf * t_half
                    oh = ohw[:, ts0:ts0 + t_half, :]

                    lr = wpool.tile([128, t_half, FE], dt.bfloat16, tag="lr")
                    # projection: 2 tiles per PSUM chunk (2 banks)
                    for half_t in range(t_half // 2):
                        pr = psPr.tile([128, 2, 512], dt.float32, tag="pr")
                        for k in range(2):
                            t = ts0 + half_t * 2 + k
                            c0 = t * 128
                            nc.tensor.matmul(
                                pr[:, k, 0:NPROJ], lhsT=nfe[:, c0:c0 + 128],
                                rhs=wcat_s[:], start=True, stop=True,
                                skip_group_check=True)
                            nc.tensor.matmul(
                                pr[:, k, 0:FE], lhsT=dfe[:, c0:c0 + 128],
                                rhs=wnj_s[:], start=False, stop=True,
                                skip_group_check=True)
                        tl = ts0 + half_t * 2
                        # stage the whole projection out of PSUM (bf16)
                        nc.scalar.copy(stg[:, tl:tl + 2, :],
                                       pr[:, :, 0:NPROJ])

                    # leaky relu, batched over the half-window
                    fo = stg[:, ts0:ts0 + t_half, 0:FE]
                    nc.vector.scalar_tensor_tensor(
                        out=lr[:], in0=fo, scalar=SLOPE, in1=fo,
                        op0=OP.mult, op1=OP.max)
                    ea = wpool.tile([128, t_half, FE], dt.bfloat16, tag="ea")
                    nc.vector.tensor_tensor(
                        out=ea[:], in0=lr[:],
                        in1=attn_s[:].unsqueeze(1).broadcast_to(
                            [128, t_half, FE]),
                        op=OP.mult)
                    eat = wpool.tile([128, t_half, H], dt.float32, tag="eat")
                    nc.vector.tensor_reduce(
                        eat[:], ea[:].rearrange("p t (h f) -> p t h f",
                                                f=OUT_EDGE),
                        axis=mybir.AxisListType.X, op=OP.add)
                    nc.scalar.activation(rhs[:, ts0:ts0 + t_half, NPAY:],
                                         eat[:], AF.Exp)
                    nc.vector.tensor_tensor(
                        out=rhs[:, ts0:ts0 + t_half, 0:NPAY]
                        .rearrange("p t (h f) -> p t h f", f=OUT_NODE),
                        in0=stg[:, ts0:ts0 + t_half, FE:NPROJ]
                        .rearrange("p t (h f) -> p t h f", f=OUT_NODE),
                        in1=rhs[:, ts0:ts0 + t_half, NPAY:].unsqueeze(3)
                        .broadcast_to([128, t_half, H, OUT_NODE]),
                        op=OP.mult)
                    for t in range(t_half):
                        tg = ts0 + t
                        nc.tensor.matmul(P[:], lhsT=oh[:, t, :],
                                         rhs=rhs[:, tg, :],
                                         start=(tg == 0), stop=(tg == t_w - 1),
                                         skip_group_check=True)

                # ---- epilogue -------------------------------------------
                sg = epool.tile([128, H], dt.float32, tag="sg")
                nc.vector.tensor_scalar(out=sg[:], in0=P[:, NPAY:],
                                        scalar1=1e-30, scalar2=None,
                                        op0=OP.max)
                si = epool.tile([128, H], dt.float32, tag="si")
                nc.vector.reciprocal(si[:], sg[:])
                tmp = epool.tile([128, OUT_NODE, H], dt.float32, tag="tmp")
                nc.vector.tensor_tensor(
                    out=tmp[:].transpose([0, 2, 1]),
                    in0=P[:, 0:NPAY].rearrange("p (h f) -> p h f", f=OUT_NODE),
                    in1=si[:].unsqueeze(2).broadcast_to([128, H, OUT_NODE]),
                    op=OP.mult)
                acc = epool.tile([128, OUT_NODE], dt.float32, tag="acc")
                nc.vector.tensor_reduce(acc[:], tmp[:],
                                        axis=mybir.AxisListType.X, op=OP.add)
                m01 = epool.tile([128, 1], dt.float32, tag="m01")
                nc.vector.tensor_scalar(out=m01[:], in0=P[:, NPAY:NPAY + 1],
                                        scalar1=0.0, scalar2=None, op0=OP.is_gt)
                acc2 = epool.tile([128, OUT_NODE], dt.float32, tag="acc2")
                nc.vector.tensor_add(acc2[:], acc[:], bmean_s[:])
                nc.vector.tensor_scalar(out=out_acc[:, w, :], in0=acc2[:],
                                        scalar1=0.0, scalar2=m01[:],
                                        op0=OP.max, op1=OP.mult)

            nc.sync.dma_start(OUT[:].rearrange("(w p) c -> p w c", p=128),
                              out_acc[:])

    if not nc.is_finalized():
        nc.finalize()
    return nc


# ===========================================================================
# numpy emulation of the device program (for validation/debug)
# ===========================================================================

def emulate_core(in_map, W, cfg):
    t_half = cfg["t_half"]
    slots = 2 * t_half * 128
    FE = H * OUT_EDGE
    NPAY = H * OUT_NODE

    f32 = np.float32
    wcat = in_map["wcat"].astype(f32)
    wnj = in_map["wnj"].astype(f32)
    attn_rep = in_map["attn_rep"][0].astype(f32)
    bmean = in_map["bmean"][0]

    out = np.zeros((W * 128, OUT_NODE), f32)
    for w in range(W):
        zfe = in_map["zfe"][:, w * 2 * slots:(w + 1) * 2 * slots].astype(f32)
        nfe = zfe[:, 0:slots].T
        dfe = zfe[:, slots:].T
        proj = nfe @ wcat                       # [slots, 320] (psum f32)
        proj[:, 0:FE] += dfe @ wnj
        pay = proj[:, FE:].astype(BF16).astype(f32)
        fout = proj[:, 0:FE].astype(BF16).astype(f32)
        lr = np.maximum(fout, SLOPE * fout).astype(BF16).astype(f32)
        eat = ((lr * attn_rep[None, :]).astype(BF16).astype(f32)
               .reshape(-1, H, OUT_EDGE).sum(axis=2))
        wgt = np.exp(eat).astype(BF16).astype(f32)          # [slots, H]
        oh = (in_map["ohm"][:, w].astype(f32).transpose(1, 0, 2)
              .reshape(slots, 128))
        rhs = np.concatenate(
            [(pay.reshape(-1, H, OUT_NODE)
              * wgt[:, :, None]).reshape(-1, NPAY).astype(BF16).astype(f32),
             wgt], axis=1)
        P = oh.T @ rhs                                       # [128, 260]
        s = np.maximum(P[:, NPAY:], 1e-30)
        acc = (P[:, 0:NPAY].reshape(128, H, OUT_NODE) /
               s[:, :, None]).sum(axis=1)
        m01 = (P[:, NPAY:NPAY + 1] > 0).astype(f32)
        out[w * 128:(w + 1) * 128] = np.maximum(acc + bmean[None, :], 0) * m01
    return out


def assemble(meta, results):
    n_dst = meta["cfg"]["n_dst"]
    out = np.zeros((n_dst, OUT_NODE), np.float32)
    for c in range(N_CORES):
        slots_rows, glob_rows = meta["asm"][c]
        if len(glob_rows):
            out[glob_rows] = results[c]["out"][slots_rows]
    return out


# ===========================================================================
# entry point
# ===========================================================================

_CACHE = {}
LAST_EXEC_NS = None
LAST_RESULT = None


def kernel(nfeats, dst_feats, reward, src, dst,
           W_ns, b_ns, W_ni, W_nj, W_fij, attn, b_e):
    global LAST_EXEC_NS, LAST_RESULT
    import os
    from concourse.bass_utils import run_bass_kernel_spmd

    meta, in_maps = prep(nfeats, dst_feats, reward, src, dst,
                         W_ns, b_ns, W_ni, W_nj, W_fij, attn, b_e)
    key = meta["W"]
    if key not in _CACHE:
        _CACHE[key] = build_program(meta["W"], meta["cfg"])
    nc = _CACHE[key]
    kwargs = {}
    if os.environ.get("EGAT_TRACE"):
        kwargs = dict(trace=True)
    try:
        res = run_bass_kernel_spmd(nc, in_maps, list(range(N_CORES)), **kwargs)
    except ModuleNotFoundError:
        # NTFF profile hook unavailable in this environment
        res = run_bass_kernel_spmd(nc, in_maps, list(range(N_CORES)))
    LAST_EXEC_NS = res.exec_time_ns
    LAST_RESULT = res
    return assemble(meta, res.results)


def estimate_ns(W=None, cfg=None):
    """Cost-model (no_exec CoreSim) estimate of the per-core kernel time."""
    from concourse.bass_interp import CoreSim
    cfg = cfg or default_cfg()
    if W is None:
        W = sorted(_CACHE)[0] if _CACHE else 50
    nc = _CACHE.get(W) or build_program(W, cfg)
    sim = CoreSim(nc, no_exec=True)
    sim.simulate()
    return int(sim.time)



# revision 14
# speedup vs baseline: 1.7638x; 1.7638x over previous
"""EGAT (edge-featured GAT) kernel for 8 Trainium2 NeuronCores.

Edge-parallel sharding: edges are sorted by dst and split into 8 contiguous
shards at dst boundaries, so each core owns a disjoint dst range end-to-end
(softmax stats + aggregation are fully local -> no collectives).

Per core the edges are packed into W windows of 2048 edge slots (16 tiles of
128).  A window covers at most 128 distinct dst nodes; dst rows are packed
DENSELY by rank within the window (row r = r-th distinct dst), so every
materialized row has >=1 edge and no empty-row gating is needed.

Device pipeline per window (engine-balanced):
  PE    : FE projection (src@W_ni + dst@W_nj -> small PSUM), payload
          projection (src@W_ns, f-major columns), one-hot scatter matmul.
  Act   : leaky_relu (Prelu table), exp, final relu (all table-set
          "exp_and_others" -> single table load).
  DVE   : attention dot (mult + reduce), part of the weight-multiply,
          epilogue normalization.
  Pool  : the other part of the weight-multiply (reads payload straight
          from PSUM -> no staging copies anywhere).
  DMA   : src/dst gathered features (zfe) on SP queue, one-hot (ohm) +
          per-window output on the Act queue.

The reward term r*colsum(W_fij) and bias b_e ride along via host-crafted
virtual feature rows x_row, y_row added to the gathered dst rows (x_row @
W_nj = colsum(W_fij), y_row @ W_nj = b_e).  b_ns enters as mean(b_ns) added
per dst in the epilogue (softmax weights sum to 1 per head).
"""

import sys

sys.path.insert(0, "/opt/trn_rl_repo")

import numpy as np
import ml_dtypes

BF16 = ml_dtypes.bfloat16

# ---- problem constants (hardcoded per the task contract) -------------------
N_SRC = 50000
N_DST = 50000
E = 800000
IN_NODE = 128
IN_EDGE = 16
OUT_NODE = 64
OUT_EDGE = 16
H = 4
SLOPE = 0.01

N_CORES = 8

FE = H * OUT_EDGE           # 64
NPAY = H * OUT_NODE         # 256
NTOT = NPAY + H             # 260 scatter cols: [pay f-major | wgt]


def default_cfg():
    return dict(n_dst=N_DST, t_half=8)


# ===========================================================================
# Host-side packing
# ===========================================================================

def prep(nfeats, dst_feats, reward, src, dst, W_ns, b_ns, W_ni, W_nj, W_fij,
         attn, b_e, cfg=None):
    """Sort/shard/pack everything. Returns (meta, in_maps)."""
    cfg = cfg or default_cfg()
    t_half = cfg["t_half"]
    t_w = 2 * t_half
    slots = t_w * 128

    e_tot = src.shape[0]

    nfeats = np.asarray(nfeats, np.float32)
    dst_feats = np.asarray(dst_feats, np.float32)
    reward = np.asarray(reward, np.float32)
    src = np.asarray(src, np.int64)
    dst = np.asarray(dst, np.int64)
    W_ns = np.asarray(W_ns, np.float32)
    b_ns = np.asarray(b_ns, np.float32)
    W_ni = np.asarray(W_ni, np.float32)
    W_nj = np.asarray(W_nj, np.float32)
    W_fij = np.asarray(W_fij, np.float32)
    attn = np.asarray(attn, np.float32)
    b_e = np.asarray(b_e, np.float32)

    # ---- sort by dst and shard at dst boundaries --------------------------
    order = np.argsort(dst, kind="stable")
    d_s = dst[order]
    s_s = src[order]
    r_s = reward[order]

    cut = [0]
    for c in range(1, N_CORES):
        t = (e_tot * c) // N_CORES
        while t < e_tot and t > 0 and d_s[t] == d_s[t - 1]:
            t += 1
        cut.append(t)
    cut.append(e_tot)

    # ---- greedy window packing per core (dense rank-based rows) -----------
    per_core = []
    for c in range(N_CORES):
        e0, e1 = cut[c], cut[c + 1]
        d = d_s[e0:e1]
        wins = []  # (w_start, w_count, n_uniq) over local positions
        if e1 > e0:
            uniq, starts = np.unique(d, return_index=True)
            ends = np.append(starts[1:], len(d))
            w_start = 0
            w_count = 0
            w_uniq = 0
            for gi in range(len(uniq)):
                glen = int(ends[gi] - starts[gi])
                if w_uniq + 1 > 128 or w_count + glen > slots:
                    wins.append((w_start, w_count, w_uniq))
                    w_start = int(starts[gi])
                    w_count = 0
                    w_uniq = 0
                w_count += glen
                w_uniq += 1
            wins.append((w_start, w_count, w_uniq))
        per_core.append((e0, e1, wins))

    W = max(1, max(len(pc[2]) for pc in per_core))

    # virtual feature rows: x_row @ W_nj == colsum(W_fij); y_row @ W_nj == b_e
    wsum = W_fij.sum(axis=0)
    x_row = np.linalg.lstsq(W_nj.T.astype(np.float64), wsum.astype(np.float64),
                            rcond=None)[0].astype(np.float32)
    y_row = np.linalg.lstsq(W_nj.T.astype(np.float64), b_e.astype(np.float64),
                            rcond=None)[0].astype(np.float32)

    mf_all = []     # [128, W, t_w, 128] bf16 one-hot (rank) per slot
    zfe_all = []    # [128, W*2*slots] bf16 (src feats | dst feats per window)
    asm = []        # per core (slot_rows, global_rows)

    for c in range(N_CORES):
        e0, e1, wins = per_core[c]
        d = d_s[e0:e1]
        s = s_s[e0:e1]
        r = r_s[e0:e1]

        drel = np.full((W, slots), -1.0, np.float32)
        nfe = np.zeros((W * slots, IN_NODE), np.float32)
        dfe = np.zeros((W * slots, IN_NODE), np.float32)
        rows_slot = []
        rows_glob = []
        for w, (ws, wc, nu) in enumerate(wins):
            sl = slice(ws, ws + wc)
            dsl = d[sl]
            uds, rank = np.unique(dsl, return_inverse=True)
            drel[w, :wc] = rank.astype(np.float32)
            nfe[w * slots:w * slots + wc] = nfeats[s[sl]]
            dfe[w * slots:w * slots + wc] = (dst_feats[dsl]
                                             + r[sl, None] * x_row[None, :]
                                             + y_row[None, :])
            rows_slot.append(w * 128 + np.arange(len(uds)))
            rows_glob.append(uds)

        # one-hot per slot, layout [128 p, W, t, 128 dcol]
        ohm = (drel.reshape(W, t_w, 128)[:, :, :, None]
               == np.arange(128, dtype=np.float32)).astype(BF16)
        ohm = np.ascontiguousarray(ohm.transpose(2, 0, 1, 3))

        zfe = np.empty((IN_NODE, W, 2 * slots), np.float32)
        zfe[:, :, :slots] = nfe.T.reshape(IN_NODE, W, slots)
        zfe[:, :, slots:] = dfe.T.reshape(IN_NODE, W, slots)
        mf_all.append(ohm)
        zfe_all.append(np.ascontiguousarray(
            zfe.reshape(IN_NODE, W * 2 * slots).astype(BF16)))
        asm.append((np.concatenate(rows_slot) if rows_slot else
                    np.zeros(0, np.int64),
                    np.concatenate(rows_glob) if rows_glob else
                    np.zeros(0, np.int64)))

    # ---- shared constants -------------------------------------------------
    # payload columns f-major: col f*H + h  <-  W_ns[:, h*OUT_NODE + f]
    wpay = (W_ns * 0.25).reshape(IN_NODE, H, OUT_NODE)   # fold mean over heads
    wpay = np.ascontiguousarray(wpay.transpose(0, 2, 1)).reshape(IN_NODE, NPAY)
    wfe = np.ascontiguousarray(W_ni).astype(BF16)                  # [128,64]
    wpay = wpay.astype(BF16)                                       # [128,256]
    wnj = W_nj.astype(BF16)                                        # [128,64]
    attn_rep = np.broadcast_to(attn.reshape(-1).astype(np.float32),
                               (128, FE)).astype(BF16).copy()
    bmean = np.broadcast_to(b_ns.reshape(H, OUT_NODE).mean(axis=0),
                            (128, OUT_NODE)).astype(np.float32).copy()

    in_maps = []
    for c in range(N_CORES):
        in_maps.append(dict(
            zfe=zfe_all[c], ohm=mf_all[c],
            wfe=wfe, wpay=wpay, wnj=wnj, attn_rep=attn_rep, bmean=bmean,
        ))

    meta = dict(W=W, asm=asm, cfg=cfg)
    return meta, in_maps


# ===========================================================================
# Device program
# ===========================================================================

def build_program(W, cfg):
    import concourse.bacc as bacc
    import concourse.tile as tile
    import concourse.mybir as mybir
    from contextlib import ExitStack

    dt = mybir.dt
    AF = mybir.ActivationFunctionType
    OP = mybir.AluOpType

    t_half = cfg["t_half"]
    t_w = 2 * t_half
    slots = t_w * 128

    nc = bacc.Bacc(None, target_bir_lowering=False)

    ZFE = nc.declare_dram_parameter("zfe", [IN_NODE, W * 2 * slots],
                                    dt.bfloat16, isOutput=False)
    OHM = nc.declare_dram_parameter("ohm", [128, W, t_w, 128], dt.bfloat16,
                                    isOutput=False)
    WFE = nc.declare_dram_parameter("wfe", [IN_NODE, FE], dt.bfloat16,
                                    isOutput=False)
    WPAY = nc.declare_dram_parameter("wpay", [IN_NODE, NPAY], dt.bfloat16,
                                     isOutput=False)
    WNJ = nc.declare_dram_parameter("wnj", [IN_NODE, FE], dt.bfloat16,
                                    isOutput=False)
    ATTN = nc.declare_dram_parameter("attn_rep", [128, FE], dt.bfloat16,
                                     isOutput=False)
    BMEAN = nc.declare_dram_parameter("bmean", [128, OUT_NODE], dt.float32,
                                      isOutput=False)
    OUT = nc.declare_dram_parameter("out", [W * 128, OUT_NODE], dt.float32,
                                    isOutput=True)

    with tile.TileContext(nc) as tc, ExitStack() as ctx:
        cpool = ctx.enter_context(tc.tile_pool(name="consts", bufs=1))
        wfe_s = cpool.tile([128, FE], dt.bfloat16)
        nc.sync.dma_start(wfe_s[:], WFE[:])
        wpay_s = cpool.tile([128, NPAY], dt.bfloat16)
        nc.sync.dma_start(wpay_s[:], WPAY[:])
        wnj_s = cpool.tile([128, FE], dt.bfloat16)
        nc.sync.dma_start(wnj_s[:], WNJ[:])
        attn_s = cpool.tile([128, FE], dt.bfloat16)
        nc.sync.dma_start(attn_s[:], ATTN[:])
        bmean_s = cpool.tile([128, OUT_NODE], dt.float32)
        nc.sync.dma_start(bmean_s[:], BMEAN[:])

        with tc.tile_pool(name="feat", bufs=3) as fpool, \
             tc.tile_pool(name="meta", bufs=3) as mpool, \
             tc.tile_pool(name="work", bufs=3) as wpool, \
             tc.tile_pool(name="stg", bufs=3) as spool, \
             tc.tile_pool(name="rhsp", bufs=3) as rpool, \
             tc.tile_pool(name="ep", bufs=2) as epool, \
             tc.tile_pool(name="psFE", bufs=2, space="PSUM") as psFE, \
             tc.tile_pool(name="psPay", bufs=2, space="PSUM") as psPay, \
             tc.tile_pool(name="psP", bufs=2, space="PSUM") as psP:
            for w in range(W):
                zfe = fpool.tile([128, 2 * slots], dt.bfloat16, tag="zfe")
                nc.sync.dma_start(zfe[:],
                                  ZFE[:, w * 2 * slots:(w + 1) * 2 * slots])
                nfe = zfe[:, 0:slots]
                dfe = zfe[:, slots:2 * slots]
                ohw = mpool.tile([128, t_w, 128], dt.bfloat16, tag="ohw")
                nc.gpsimd.dma_start(ohw[:, 0:t_half, :],
                                    OHM[:, w, 0:t_half, :])
                nc.sync.dma_start(ohw[:, t_half:, :],
                                  OHM[:, w, t_half:, :])

                rhs = rpool.tile([128, t_w, NTOT], dt.bfloat16, tag="rhs")
                ea = wpool.tile([128, t_w, FE], dt.bfloat16, tag="ea")
                eat = wpool.tile([128, t_w, H], dt.float32, tag="eat")
                P = psP.tile([128, NTOT], dt.float32, tag="P")

                for hf in range(2):
                    ts0 = hf * t_half
                    # ---- FE projection: src@W_ni + dst@W_nj --------------
                    prFE = psFE.tile([128, t_half, FE], dt.float32, tag="prFE")
                    for t in range(t_half):
                        c0 = (ts0 + t) * 128
                        nc.tensor.matmul(prFE[:, t, :],
                                         lhsT=nfe[:, c0:c0 + 128],
                                         rhs=wfe_s[:], start=True, stop=False,
                                         skip_group_check=True)
                        nc.tensor.matmul(prFE[:, t, :],
                                         lhsT=dfe[:, c0:c0 + 128],
                                         rhs=wnj_s[:], start=False, stop=True,
                                         skip_group_check=True)
                    # ---- leaky relu on Act (Prelu alpha=slope) -----------
                    lr = wpool.tile([128, t_half, FE], dt.bfloat16, tag="lr")
                    nc.scalar.activation(lr[:], prFE[:], AF.Prelu, alpha=SLOPE)
                    # ---- attention dot: mult on Pool (reduce batched later)
                    nc.gpsimd.tensor_tensor(
                        out=ea[:, ts0:ts0 + t_half, :], in0=lr[:],
                        in1=attn_s[:].unsqueeze(1).broadcast_to(
                            [128, t_half, FE]),
                        op=OP.mult)
                    nc.vector.tensor_reduce(
                        eat[:, ts0:ts0 + t_half, :],
                        ea[:, ts0:ts0 + t_half, :]
                        .rearrange("p t (h f) -> p t h f", f=OUT_EDGE),
                        axis=mybir.AxisListType.X, op=OP.add)
                    # ---- exp -> wgt columns of rhs -----------------------
                    nc.scalar.activation(rhs[:, ts0:ts0 + t_half, NPAY:],
                                         eat[:, ts0:ts0 + t_half, :], AF.Exp)

                    # ---- payload projection + weight-mult, 4-tile chunks -
                    # one chunk per half goes straight through DVE (PSUM
                    # read); the other is staged to SBUF by Act then
                    # weighted on Pool (GPSIMD cannot touch PSUM).
                    for cch in range(t_half // 4):
                        tb = ts0 + cch * 4
                        prP = psPay.tile([128, 4, NPAY], dt.float32, tag="prP")
                        for k in range(4):
                            c0 = (tb + k) * 128
                            nc.tensor.matmul(prP[:, k, :],
                                             lhsT=nfe[:, c0:c0 + 128],
                                             rhs=wpay_s[:], start=True,
                                             stop=True, skip_group_check=True)
                        def wmul_direct(lo, hi):
                            nc.vector.tensor_tensor(
                                out=rhs[:, tb + lo:tb + hi, 0:NPAY]
                                .rearrange("p t (f h) -> p t f h", h=H),
                                in0=prP[:, lo:hi, :]
                                .rearrange("p t (f h) -> p t f h", h=H),
                                in1=rhs[:, tb + lo:tb + hi, NPAY:]
                                .unsqueeze(2)
                                .broadcast_to([128, hi - lo, OUT_NODE, H]),
                                op=OP.mult)

                        def wmul_staged(lo, hi):
                            stg = spool.tile([128, 4, NPAY], dt.bfloat16,
                                             tag="stg")
                            nc.scalar.copy(stg[:, lo:hi, :], prP[:, lo:hi, :])
                            nc.gpsimd.tensor_tensor(
                                out=rhs[:, tb + lo:tb + hi, 0:NPAY]
                                .rearrange("p t (f h) -> p t f h", h=H),
                                in0=stg[:, lo:hi, :]
                                .rearrange("p t (f h) -> p t f h", h=H),
                                in1=rhs[:, tb + lo:tb + hi, NPAY:]
                                .unsqueeze(2)
                                .broadcast_to([128, hi - lo, OUT_NODE, H]),
                                op=OP.mult)

                        # 1.5 chunks direct on DVE, 2.5 staged (Act -> Pool)
                        if cch == 0 and hf == 0:
                            wmul_direct(0, 4)
                        elif cch == 0 and hf == 1:
                            wmul_direct(0, 2)
                            wmul_staged(2, 4)
                        else:
                            wmul_staged(0, 4)

                # ---- one-hot scatter matmul -----------------------------
                for t in range(t_w):
                    nc.tensor.matmul(P[:], lhsT=ohw[:, t, :],
                                     rhs=rhs[:, t, :],
                                     start=(t == 0), stop=(t == t_w - 1),
                                     skip_group_check=True)

                # ---- epilogue -------------------------------------------
                sg = epool.tile([128, H], dt.float32, tag="sg")
                nc.vector.tensor_scalar(out=sg[:], in0=P[:, NPAY:],
                                        scalar1=1e-30, scalar2=None,
                                        op0=OP.max)
                si = epool.tile([128, H], dt.float32, tag="si")
                nc.vector.reciprocal(si[:], sg[:])
                tmp = epool.tile([128, OUT_NODE, H], dt.float32, tag="tmp")
                nc.vector.tensor_tensor(
                    out=tmp[:],
                    in0=P[:, 0:NPAY].rearrange("p (f h) -> p f h", h=H),
                    in1=si[:].unsqueeze(1).broadcast_to([128, OUT_NODE, H]),
                    op=OP.mult)
                acc = epool.tile([128, OUT_NODE], dt.float32, tag="acc")
                nc.vector.tensor_reduce(acc[:], tmp[:],
                                        axis=mybir.AxisListType.X, op=OP.add)
                acc2 = epool.tile([128, OUT_NODE], dt.float32, tag="acc2")
                nc.vector.tensor_add(acc2[:], acc[:], bmean_s[:])
                if w % 4 == 0:
                    res = epool.tile([128, 4, OUT_NODE], dt.float32,
                                     tag="res")
                nc.scalar.activation(res[:, w % 4, :], acc2[:], AF.Relu)
                if w % 4 == 3 or w == W - 1:
                    w0 = (w // 4) * 4
                    nc.sync.dma_start(
                        OUT[w0 * 128:(w + 1) * 128, :]
                        .rearrange("(b p) c -> p b c", p=128),
                        res[:, 0:w - w0 + 1, :])

    if not nc.is_finalized():
        nc.finalize()
    return nc


# ===========================================================================
# numpy emulation of the device program (for validation/debug)
# ===========================================================================

def emulate_core(in_map, W, cfg):
    t_half = cfg["t_half"]
    slots = 2 * t_half * 128

    f32 = np.float32
    wfe = in_map["wfe"].astype(f32)
    wpay = in_map["wpay"].astype(f32)
    wnj = in_map["wnj"].astype(f32)
    attn_rep = in_map["attn_rep"][0].astype(f32)
    bmean = in_map["bmean"][0]

    out = np.zeros((W * 128, OUT_NODE), f32)
    for w in range(W):
        zfe = in_map["zfe"][:, w * 2 * slots:(w + 1) * 2 * slots].astype(f32)
        nfe = zfe[:, 0:slots].T
        dfe = zfe[:, slots:].T
        fout = (nfe @ wfe + dfe @ wnj)                   # [slots, 64] psum f32
        lr = np.where(fout > 0, fout, SLOPE * fout).astype(BF16).astype(f32)
        ea = (lr * attn_rep[None, :]).astype(BF16).astype(f32)
        eat = ea.reshape(-1, H, OUT_EDGE).sum(axis=2)    # f32
        wgt = np.exp(eat).astype(BF16).astype(f32)       # [slots, H]
        pay = (nfe @ wpay)                               # [slots, 256] f-major
        rhs_pay = (pay.reshape(-1, OUT_NODE, H)
                   * wgt[:, None, :]).reshape(-1, NPAY).astype(BF16).astype(f32)
        oh = (in_map["ohm"][:, w].astype(f32).transpose(1, 0, 2)
              .reshape(slots, 128))
        P = oh.T @ np.concatenate([rhs_pay, wgt], axis=1)   # [128, 260]
        s = np.maximum(P[:, NPAY:], 1e-30)
        si = 1.0 / s
        acc = (P[:, 0:NPAY].reshape(128, OUT_NODE, H) * si[:, None, :]
               ).sum(axis=2)
        out[w * 128:(w + 1) * 128] = np.maximum(acc + bmean[None, :], 0)
    return out


def assemble(meta, results):
    n_dst = meta["cfg"]["n_dst"]
    out = np.zeros((n_dst, OUT_NODE), np.float32)
    for c in range(N_CORES):
        slots_rows, glob_rows = meta["asm"][c]
        if len(glob_rows):
            out[glob_rows] = results[c]["out"][slots_rows]
    return out


# ===========================================================================
# entry point
# ===========================================================================

_CACHE = {}
LAST_EXEC_NS = None
LAST_RESULT = None


def kernel(nfeats, dst_feats, reward, src, dst,
           W_ns, b_ns, W_ni, W_nj, W_fij, attn, b_e):
    global LAST_EXEC_NS, LAST_RESULT
    import os
    from concourse.bass_utils import run_bass_kernel_spmd

    meta, in_maps = prep(nfeats, dst_feats, reward, src, dst,
                         W_ns, b_ns, W_ni, W_nj, W_fij, attn, b_e)
    key = meta["W"]
    if key not in _CACHE:
        _CACHE[key] = build_program(meta["W"], meta["cfg"])
    nc = _CACHE[key]
    kwargs = {}
    if os.environ.get("EGAT_TRACE"):
        kwargs = dict(trace=True)
    try:
        res = run_bass_kernel_spmd(nc, in_maps, list(range(N_CORES)), **kwargs)
    except ModuleNotFoundError:
        # NTFF profile hook unavailable in this environment
        res = run_bass_kernel_spmd(nc, in_maps, list(range(N_CORES)))
    LAST_EXEC_NS = res.exec_time_ns
    LAST_RESULT = res
    return assemble(meta, res.results)


def estimate_ns(W=None, cfg=None):
    """Cost-model (no_exec CoreSim) estimate of the per-core kernel time."""
    from concourse.bass_interp import CoreSim
    cfg = cfg or default_cfg()
    if W is None:
        W = sorted(_CACHE)[0] if _CACHE else 50
    nc = _CACHE.get(W) or build_program(W, cfg)
    sim = CoreSim(nc, no_exec=True)
    sim.simulate()
    return int(sim.time)


# revision 56
# speedup vs baseline: 1.8089x; 1.0256x over previous
"""EGAT (edge-featured GAT) kernel for 8 Trainium2 NeuronCores.

Edge-parallel sharding: edges are sorted by dst and split into 8 contiguous
shards at dst boundaries, so each core owns a disjoint dst range end-to-end
(softmax stats + aggregation are fully local -> no collectives).

Per core the edges are packed into W windows of 2048 edge slots (16 tiles of
128).  A window covers at most 128 distinct dst nodes; dst rows are packed
DENSELY by rank within the window (row r = r-th distinct dst), so every
materialized row has >=1 edge and no empty-row gating is needed.

Device pipeline per window (engine-balanced):
  PE    : FE projection (src@W_ni + dst@W_nj -> small PSUM), payload
          projection (src@W_ns, f-major columns), one-hot scatter matmul.
  Act   : leaky_relu (Prelu table), exp, final relu (all table-set
          "exp_and_others" -> single table load).
  DVE   : attention dot (mult + reduce), part of the weight-multiply,
          epilogue normalization.
  Pool  : the other part of the weight-multiply (reads payload straight
          from PSUM -> no staging copies anywhere).
  DMA   : src/dst gathered features (zfe) on SP queue, one-hot (ohm) +
          per-window output on the Act queue.

The reward term r*colsum(W_fij) and bias b_e ride along via host-crafted
virtual feature rows x_row, y_row added to the gathered dst rows (x_row @
W_nj = colsum(W_fij), y_row @ W_nj = b_e).  b_ns enters as mean(b_ns) added
per dst in the epilogue (softmax weights sum to 1 per head).
"""

import sys

sys.path.insert(0, "/opt/trn_rl_repo")

import numpy as np
import ml_dtypes

BF16 = ml_dtypes.bfloat16

# ---- problem constants (hardcoded per the task contract) -------------------
N_SRC = 50000
N_DST = 50000
E = 800000
IN_NODE = 128
IN_EDGE = 16
OUT_NODE = 64
OUT_EDGE = 16
H = 4
SLOPE = 0.01

N_CORES = 8

FE = H * OUT_EDGE           # 64
NPAY = H * OUT_NODE         # 256
NTOT = NPAY + H             # 260 scatter cols: [pay f-major | wgt]


def default_cfg():
    return dict(n_dst=N_DST, t_half=8)


# ===========================================================================
# Host-side packing
# ===========================================================================

def prep(nfeats, dst_feats, reward, src, dst, W_ns, b_ns, W_ni, W_nj, W_fij,
         attn, b_e, cfg=None):
    """Sort/shard/pack everything. Returns (meta, in_maps)."""
    cfg = cfg or default_cfg()
    t_half = cfg["t_half"]
    t_w = 2 * t_half
    slots = t_w * 128

    e_tot = src.shape[0]

    nfeats = np.asarray(nfeats, np.float32)
    dst_feats = np.asarray(dst_feats, np.float32)
    reward = np.asarray(reward, np.float32)
    src = np.asarray(src, np.int64)
    dst = np.asarray(dst, np.int64)
    W_ns = np.asarray(W_ns, np.float32)
    b_ns = np.asarray(b_ns, np.float32)
    W_ni = np.asarray(W_ni, np.float32)
    W_nj = np.asarray(W_nj, np.float32)
    W_fij = np.asarray(W_fij, np.float32)
    attn = np.asarray(attn, np.float32)
    b_e = np.asarray(b_e, np.float32)

    # ---- sort by dst and shard at dst boundaries --------------------------
    order = np.argsort(dst, kind="stable")
    d_s = dst[order]
    s_s = src[order]
    r_s = reward[order]

    cut = [0]
    for c in range(1, N_CORES):
        t = (e_tot * c) // N_CORES
        while t < e_tot and t > 0 and d_s[t] == d_s[t - 1]:
            t += 1
        cut.append(t)
    cut.append(e_tot)

    # ---- greedy window packing per core (dense rank-based rows) -----------
    per_core = []
    for c in range(N_CORES):
        e0, e1 = cut[c], cut[c + 1]
        d = d_s[e0:e1]
        wins = []  # (w_start, w_count, n_uniq) over local positions
        if e1 > e0:
            uniq, starts = np.unique(d, return_index=True)
            ends = np.append(starts[1:], len(d))
            w_start = 0
            w_count = 0
            w_uniq = 0
            for gi in range(len(uniq)):
                glen = int(ends[gi] - starts[gi])
                if w_uniq + 1 > 128 or w_count + glen > slots:
                    wins.append((w_start, w_count, w_uniq))
                    w_start = int(starts[gi])
                    w_count = 0
                    w_uniq = 0
                w_count += glen
                w_uniq += 1
            wins.append((w_start, w_count, w_uniq))
        per_core.append((e0, e1, wins))

    W = max(1, max(len(pc[2]) for pc in per_core))

    # virtual feature rows: x_row @ W_nj == colsum(W_fij); y_row @ W_nj == b_e
    wsum = W_fij.sum(axis=0)
    x_row = np.linalg.lstsq(W_nj.T.astype(np.float64), wsum.astype(np.float64),
                            rcond=None)[0].astype(np.float32)
    y_row = np.linalg.lstsq(W_nj.T.astype(np.float64), b_e.astype(np.float64),
                            rcond=None)[0].astype(np.float32)

    F8 = np.dtype("float8_e4m3")
    mf_all = []     # [128, W, t_w, 128] bf16 one-hot (rank) per slot
    zfs_all = []    # [128, W*slots] bf16 src feats (payload path)
    zf8_all = []    # [64, 2, W*2*slots] fp8 src|dst feats (FE DoubleRow path)
    asm = []        # per core (slot_rows, global_rows)

    for c in range(N_CORES):
        e0, e1, wins = per_core[c]
        d = d_s[e0:e1]
        s = s_s[e0:e1]
        r = r_s[e0:e1]

        drel = np.full((W, slots), -1.0, np.float32)
        nfe = np.zeros((W * slots, IN_NODE), np.float32)
        dfe = np.zeros((W * slots, IN_NODE), np.float32)
        rows_slot = []
        rows_glob = []
        for w, (ws, wc, nu) in enumerate(wins):
            sl = slice(ws, ws + wc)
            dsl = d[sl]
            uds, rank = np.unique(dsl, return_inverse=True)
            drel[w, :wc] = rank.astype(np.float32)
            nfe[w * slots:w * slots + wc] = nfeats[s[sl]]
            dfe[w * slots:w * slots + wc] = (dst_feats[dsl]
                                             + r[sl, None] * x_row[None, :]
                                             + y_row[None, :])
            rows_slot.append(w * 128 + np.arange(len(uds)))
            rows_glob.append(uds)

        # one-hot per slot, layout [128 p, W, t, 128 dcol]
        ohm = (drel.reshape(W, t_w, 128)[:, :, :, None]
               == np.arange(128, dtype=np.float32)).astype(BF16)
        ohm = np.ascontiguousarray(ohm.transpose(2, 0, 1, 3))

        zfs = np.ascontiguousarray(
            nfe.T.reshape(IN_NODE, W * slots).astype(BF16))
        # fp8 DoubleRow pack [128, 2, W*slots]: j=0 src feats, j=1 dst feats
        # (one DR matmul contracts over 256 = 128 src-feat + 128 dst-feat)
        zf8 = np.empty((IN_NODE, 2, W * slots), np.float32)
        zf8[:, 0, :] = nfe.T.reshape(IN_NODE, W * slots)
        zf8[:, 1, :] = dfe.T.reshape(IN_NODE, W * slots)
        mf_all.append(ohm.astype(F8))
        zfs_all.append(zfs)
        zf8_all.append(np.ascontiguousarray(zf8.astype(F8)))
        asm.append((np.concatenate(rows_slot) if rows_slot else
                    np.zeros(0, np.int64),
                    np.concatenate(rows_glob) if rows_glob else
                    np.zeros(0, np.int64)))

    # ---- shared constants -------------------------------------------------
    # payload columns f-major: col f*H + h  <-  W_ns[:, h*OUT_NODE + f]
    wpay = (W_ns * 0.25).reshape(IN_NODE, H, OUT_NODE)   # fold mean over heads
    wpay = np.ascontiguousarray(wpay.transpose(0, 2, 1)).reshape(IN_NODE, NPAY)
    wpay = wpay.astype(BF16)                                       # [128,256]
    # FE weights fp8 DoubleRow-packed [128, 2, FE]: j=0 W_ni, j=1 W_nj
    wfej8 = np.empty((IN_NODE, 2, FE), np.float32)
    wfej8[:, 0, :] = W_ni
    wfej8[:, 1, :] = W_nj
    wfej8 = np.ascontiguousarray(wfej8.astype(F8))
    # block-diag attention for the PE dot: attnblk[h*16+f, h'] = attn[h,f]
    # if h'==h else 0
    blk = np.zeros((FE, H), np.float32)
    for h in range(H):
        blk[h * OUT_EDGE:(h + 1) * OUT_EDGE, h] = attn[h]
    attnblk = np.ascontiguousarray(
        np.concatenate([blk, blk], axis=0).astype(BF16))   # [128, 4]
    bmean = np.broadcast_to(b_ns.reshape(H, OUT_NODE).mean(axis=0),
                            (128, OUT_NODE)).astype(np.float32).copy()

    in_maps = []
    for c in range(N_CORES):
        in_maps.append(dict(
            zfs=zfs_all[c], zf8=zf8_all[c], ohm=mf_all[c],
            wfej8=wfej8, wpay=wpay, attnblk=attnblk, bmean=bmean,
        ))

    meta = dict(W=W, asm=asm, cfg=cfg)
    return meta, in_maps


# ===========================================================================
# Device program
# ===========================================================================

def build_program(W, cfg):
    import concourse.bacc as bacc
    import concourse.tile as tile
    import concourse.mybir as mybir
    from contextlib import ExitStack

    dt = mybir.dt
    AF = mybir.ActivationFunctionType
    OP = mybir.AluOpType

    t_half = cfg["t_half"]
    t_w = 2 * t_half
    slots = t_w * 128

    nc = bacc.Bacc(None, target_bir_lowering=False)

    ZFS = nc.declare_dram_parameter("zfs", [IN_NODE, W * slots],
                                    dt.bfloat16, isOutput=False)
    ZF8 = nc.declare_dram_parameter("zf8", [IN_NODE, 2, W * slots],
                                    dt.float8e4, isOutput=False)
    OHM = nc.declare_dram_parameter("ohm", [128, W, t_w, 128], dt.float8e4,
                                    isOutput=False)
    WFEJ8 = nc.declare_dram_parameter("wfej8", [IN_NODE, 2, FE], dt.float8e4,
                                      isOutput=False)
    WPAY = nc.declare_dram_parameter("wpay", [IN_NODE, NPAY], dt.bfloat16,
                                     isOutput=False)
    ATTN = nc.declare_dram_parameter("attnblk", [128, H], dt.bfloat16,
                                     isOutput=False)
    BMEAN = nc.declare_dram_parameter("bmean", [128, OUT_NODE], dt.float32,
                                      isOutput=False)
    OUT = nc.declare_dram_parameter("out", [W * 128, OUT_NODE], dt.float32,
                                    isOutput=True)

    with tile.TileContext(nc) as tc, ExitStack() as ctx:
        cpool = ctx.enter_context(tc.tile_pool(name="consts", bufs=1))
        wfe_s = cpool.tile([128, 2, FE], dt.float8e4)
        nc.sync.dma_start(wfe_s[:], WFEJ8[:])
        wpay_s = cpool.tile([128, NPAY], dt.bfloat16)
        nc.sync.dma_start(wpay_s[:], WPAY[:])
        attn_s = cpool.tile([128, H], dt.bfloat16)
        nc.sync.dma_start(attn_s[:], ATTN[:])
        bmean_s = cpool.tile([128, OUT_NODE], dt.float32)
        nc.sync.dma_start(bmean_s[:], BMEAN[:])

        with tc.tile_pool(name="feat", bufs=4) as fpool, \
             tc.tile_pool(name="meta", bufs=4) as mpool, \
             tc.tile_pool(name="work", bufs=4) as wpool, \
             tc.tile_pool(name="stg", bufs=4) as spool, \
             tc.tile_pool(name="rhsp", bufs=4) as rpool, \
             tc.tile_pool(name="ep", bufs=3) as epool, \
             tc.tile_pool(name="psFE", bufs=2, space="PSUM") as psFE, \
             tc.tile_pool(name="psPay", bufs=3, space="PSUM") as psPay, \
             tc.tile_pool(name="psE", bufs=1, space="PSUM") as psE, \
             tc.tile_pool(name="psP", bufs=2, space="PSUM") as psP:
            for w in range(W):
                zfs = fpool.tile([128, slots], dt.bfloat16, tag="zfs")
                nc.sync.dma_start(zfs[:], ZFS[:, w * slots:(w + 1) * slots])
                zf8 = fpool.tile([128, 2, slots], dt.float8e4, tag="zf8")
                nc.sync.dma_start(zf8[:],
                                  ZF8[:, :, w * slots:(w + 1) * slots])
                ohw = mpool.tile([128, t_w, 128], dt.float8e4, tag="ohw")
                nc.gpsimd.dma_start(ohw[:], OHM[:, w, :, :])

                rhs = rpool.tile([128, t_w, NTOT], dt.bfloat16, tag="rhs")
                eatp = psE.tile([128, t_w, H], dt.float32, tag="eatp")
                P = psP.tile([128, NTOT], dt.float32, tag="P")

                for hf in range(2):
                    ts0 = hf * t_half
                    hb = ts0 * 128          # first slot of the half
                    # ---- FE projection, FE-major quarters: out[fe, slot]
                    # for 512 slots per quarter, written at partition base 0
                    # (HW requires DoubleRow output at partition 0).  One DR
                    # matmul contracts 128 src + 128 dst features.  Leaky:
                    # quarter 0 on Act (Prelu), quarter 1 on DVE (STT).
                    half_sl = t_half * 64   # 512
                    lrq = []
                    for b in range(2):
                        s0 = hb + b * half_sl
                        prFE = psFE.tile([64, t_half * 64], dt.float32,
                                         tag="prFE")
                        nc.tensor.matmul(prFE[:],
                                         lhsT=wfe_s[:],
                                         rhs=zf8[:, :, s0:s0 + half_sl],
                                         perf_mode=mybir.MatmulPerfMode
                                         .DoubleRow,
                                         start=True, stop=True,
                                         skip_group_check=True)
                        lr = wpool.tile([64, t_half * 64], dt.bfloat16,
                                        tag="lr")
                        nc.scalar.activation(lr[:], prFE[:], AF.Prelu,
                                             alpha=SLOPE)
                        lrq.append(lr)
                    # ---- attention dot on PE: e[slot,h] = lr_fe . attnblk
                    for t in range(t_half):
                        b = t // 4
                        c0 = (t - b * 4) * 128
                        nc.tensor.matmul(
                            eatp[:, ts0 + t, :],
                            lhsT=lrq[b][:, c0:c0 + 128],
                            rhs=attn_s[0:64, :],
                            start=True, stop=True, skip_group_check=True)
                    # ---- exp -> wgt columns of rhs -----------------------
                    nc.scalar.activation(rhs[:, ts0:ts0 + t_half, NPAY:],
                                         eatp[:, ts0:ts0 + t_half, :], AF.Exp)

                    # ---- payload projection + weight-mult, 4-tile chunks -
                    # one chunk per half goes straight through DVE (PSUM
                    # read); the other is staged to SBUF by Act then
                    # weighted on Pool (GPSIMD cannot touch PSUM).
                    for cch in range(t_half // 2):
                        tb = ts0 + cch * 2
                        prP = psPay.tile([128, 2, NPAY], dt.float32, tag="prP")
                        for k in range(2):
                            c0 = (tb + k) * 128
                            nc.tensor.matmul(prP[:, k, :],
                                             lhsT=zfs[:, c0:c0 + 128],
                                             rhs=wpay_s[:], start=True,
                                             stop=True, skip_group_check=True)
                        def wmul_direct(lo, hi):
                            nc.vector.tensor_tensor(
                                out=rhs[:, tb + lo:tb + hi, 0:NPAY]
                                .rearrange("p t (f h) -> p t f h", h=H),
                                in0=prP[:, lo:hi, :]
                                .rearrange("p t (f h) -> p t f h", h=H),
                                in1=rhs[:, tb + lo:tb + hi, NPAY:]
                                .unsqueeze(2)
                                .broadcast_to([128, hi - lo, OUT_NODE, H]),
                                op=OP.mult)

                        def wmul_staged(lo, hi):
                            stg = spool.tile([128, 2, NPAY], dt.bfloat16,
                                             tag="stg")
                            nc.scalar.copy(stg[:, lo:hi, :], prP[:, lo:hi, :])
                            nc.gpsimd.tensor_tensor(
                                out=rhs[:, tb + lo:tb + hi, 0:NPAY]
                                .rearrange("p t (f h) -> p t f h", h=H),
                                in0=stg[:, lo:hi, :]
                                .rearrange("p t (f h) -> p t f h", h=H),
                                in1=rhs[:, tb + lo:tb + hi, NPAY:]
                                .unsqueeze(2)
                                .broadcast_to([128, hi - lo, OUT_NODE, H]),
                                op=OP.mult)

                        # 3 chunks staged (Act -> Pool), 5 direct on DVE;
                        # later chunks direct (short path so the scatter
                        # isn't stalled behind the Act -> Pool chain)
                        if cch < (2 if hf == 0 else 1):
                            wmul_staged(0, 2)
                        else:
                            wmul_direct(0, 2)

                # ---- one-hot scatter matmul -----------------------------
                for t in range(t_w):
                    nc.tensor.matmul(P[:], lhsT=ohw[:, t, :],
                                     rhs=rhs[:, t, :],
                                     start=(t == 0), stop=(t == t_w - 1),
                                     skip_group_check=True)

                # ---- epilogue -------------------------------------------
                sg = epool.tile([128, H], dt.float32, tag="sg")
                nc.vector.tensor_scalar(out=sg[:], in0=P[:, NPAY:],
                                        scalar1=1e-30, scalar2=None,
                                        op0=OP.max)
                si = epool.tile([128, H], dt.float32, tag="si")
                nc.vector.reciprocal(si[:], sg[:])
                tmp = epool.tile([128, OUT_NODE, H], dt.float32, tag="tmp")
                nc.vector.tensor_tensor(
                    out=tmp[:],
                    in0=P[:, 0:NPAY].rearrange("p (f h) -> p f h", h=H),
                    in1=si[:].unsqueeze(1).broadcast_to([128, OUT_NODE, H]),
                    op=OP.mult)
                acc = epool.tile([128, OUT_NODE], dt.float32, tag="acc")
                nc.vector.tensor_reduce(acc[:], tmp[:],
                                        axis=mybir.AxisListType.X, op=OP.add)
                acc2 = epool.tile([128, OUT_NODE], dt.float32, tag="acc2")
                nc.gpsimd.tensor_add(acc2[:], acc[:], bmean_s[:])
                if w % 4 == 0:
                    res = epool.tile([128, 4, OUT_NODE], dt.float32,
                                     tag="res")
                nc.gpsimd.tensor_scalar(out=res[:, w % 4, :], in0=acc2[:],
                                        scalar1=0.0, scalar2=None, op0=OP.max)
                if w % 4 == 3 or w == W - 1:
                    w0 = (w // 4) * 4
                    nc.sync.dma_start(
                        OUT[w0 * 128:(w + 1) * 128, :]
                        .rearrange("(b p) c -> p b c", p=128),
                        res[:, 0:w - w0 + 1, :])

    if not nc.is_finalized():
        nc.finalize()
    return nc


# ===========================================================================
# numpy emulation of the device program (for validation/debug)
# ===========================================================================

def emulate_core(in_map, W, cfg):
    t_half = cfg["t_half"]
    slots = 2 * t_half * 128

    f32 = np.float32
    wfe = in_map["wfej8"][:, 0, :].astype(f32)
    wnj = in_map["wfej8"][:, 1, :].astype(f32)
    wpay = in_map["wpay"].astype(f32)
    attnblk = in_map["attnblk"][0:FE, :].astype(f32)     # [64, 4]
    bmean = in_map["bmean"][0]

    out = np.zeros((W * 128, OUT_NODE), f32)
    for w in range(W):
        zf8 = in_map["zf8"][:, :, w * slots:(w + 1) * slots].astype(f32)
        nf8 = zf8[:, 0, :].T
        df8 = zf8[:, 1, :].T
        nfe = in_map["zfs"][:, w * slots:(w + 1) * slots].astype(f32).T
        fout = (nf8 @ wfe + df8 @ wnj)                   # [slots, 64] psum f32
        lr = np.where(fout > 0, fout, SLOPE * fout).astype(BF16).astype(f32)
        eat = lr @ attnblk                               # PE dot, psum f32
        wgt = np.exp(eat).astype(BF16).astype(f32)       # [slots, H]
        pay = (nfe @ wpay)                               # [slots, 256] f-major
        rhs_pay = (pay.reshape(-1, OUT_NODE, H)
                   * wgt[:, None, :]).reshape(-1, NPAY).astype(BF16).astype(f32)
        oh = (in_map["ohm"][:, w].astype(f32).transpose(1, 0, 2)
              .reshape(slots, 128))
        P = oh.T @ np.concatenate([rhs_pay, wgt], axis=1)   # [128, 260]
        s = np.maximum(P[:, NPAY:], 1e-30)
        si = 1.0 / s
        acc = (P[:, 0:NPAY].reshape(128, OUT_NODE, H) * si[:, None, :]
               ).sum(axis=2)
        out[w * 128:(w + 1) * 128] = np.maximum(acc + bmean[None, :], 0)
    return out


def assemble(meta, results):
    n_dst = meta["cfg"]["n_dst"]
    out = np.zeros((n_dst, OUT_NODE), np.float32)
    for c in range(N_CORES):
        slots_rows, glob_rows = meta["asm"][c]
        if len(glob_rows):
            out[glob_rows] = results[c]["out"][slots_rows]
    return out


# ===========================================================================
# entry point
# ===========================================================================

_CACHE = {}
LAST_EXEC_NS = None
LAST_RESULT = None


def kernel(nfeats, dst_feats, reward, src, dst,
           W_ns, b_ns, W_ni, W_nj, W_fij, attn, b_e):
    global LAST_EXEC_NS, LAST_RESULT
    import os
    from concourse.bass_utils import run_bass_kernel_spmd

    meta, in_maps = prep(nfeats, dst_feats, reward, src, dst,
                         W_ns, b_ns, W_ni, W_nj, W_fij, attn, b_e)
    key = meta["W"]
    if key not in _CACHE:
        _CACHE[key] = build_program(meta["W"], meta["cfg"])
    nc = _CACHE[key]
    kwargs = {}
    if os.environ.get("EGAT_TRACE"):
        kwargs = dict(trace=True)
    try:
        res = run_bass_kernel_spmd(nc, in_maps, list(range(N_CORES)), **kwargs)
    except ModuleNotFoundError:
        # NTFF profile hook unavailable in this environment
        res = run_bass_kernel_spmd(nc, in_maps, list(range(N_CORES)))
    LAST_EXEC_NS = res.exec_time_ns
    LAST_RESULT = res
    return assemble(meta, res.results)


def estimate_ns(W=None, cfg=None):
    """Cost-model (no_exec CoreSim) estimate of the per-core kernel time."""
    from concourse.bass_interp import CoreSim
    cfg = cfg or default_cfg()
    if W is None:
        W = sorted(_CACHE)[0] if _CACHE else 50
    nc = _CACHE.get(W) or build_program(W, cfg)
    sim = CoreSim(nc, no_exec=True)
    sim.simulate()
    return int(sim.time)


# revision 63
# speedup vs baseline: 1.8834x; 1.0412x over previous
"""EGAT (edge-featured GAT) kernel for 8 Trainium2 NeuronCores.

Edge-parallel sharding: edges are sorted by dst and split into 8 contiguous
shards at dst boundaries, so each core owns a disjoint dst range end-to-end
(softmax stats + aggregation are fully local -> no collectives).

Per core the edges are packed into W windows of 2048 edge slots (16 tiles of
128).  A window covers at most 128 distinct dst nodes; dst rows are packed
DENSELY by rank within the window (row r = r-th distinct dst), so every
materialized row has >=1 edge and no empty-row gating is needed.

Device pipeline per window (engine-balanced):
  PE    : FE projection (src@W_ni + dst@W_nj -> small PSUM), payload
          projection (src@W_ns, f-major columns), one-hot scatter matmul.
  Act   : leaky_relu (Prelu table), exp, final relu (all table-set
          "exp_and_others" -> single table load).
  DVE   : attention dot (mult + reduce), part of the weight-multiply,
          epilogue normalization.
  Pool  : the other part of the weight-multiply (reads payload straight
          from PSUM -> no staging copies anywhere).
  DMA   : src/dst gathered features (zfe) on SP queue, one-hot (ohm) +
          per-window output on the Act queue.

The reward term r*colsum(W_fij) and bias b_e ride along via host-crafted
virtual feature rows x_row, y_row added to the gathered dst rows (x_row @
W_nj = colsum(W_fij), y_row @ W_nj = b_e).  b_ns enters as mean(b_ns) added
per dst in the epilogue (softmax weights sum to 1 per head).
"""

import sys

sys.path.insert(0, "/opt/trn_rl_repo")

import numpy as np
import ml_dtypes

BF16 = ml_dtypes.bfloat16

# ---- problem constants (hardcoded per the task contract) -------------------
N_SRC = 50000
N_DST = 50000
E = 800000
IN_NODE = 128
IN_EDGE = 16
OUT_NODE = 64
OUT_EDGE = 16
H = 4
SLOPE = 0.01

N_CORES = 8

FE = H * OUT_EDGE           # 64
NPAY = H * OUT_NODE         # 256
NTOT = NPAY + H             # 260 scatter cols: [pay f-major | wgt]


def default_cfg():
    return dict(n_dst=N_DST, t_half=8)


# ===========================================================================
# Host-side packing
# ===========================================================================

def prep(nfeats, dst_feats, reward, src, dst, W_ns, b_ns, W_ni, W_nj, W_fij,
         attn, b_e, cfg=None):
    """Sort/shard/pack everything. Returns (meta, in_maps)."""
    cfg = cfg or default_cfg()
    t_half = cfg["t_half"]
    t_w = 2 * t_half
    slots = t_w * 128

    e_tot = src.shape[0]

    nfeats = np.asarray(nfeats, np.float32)
    dst_feats = np.asarray(dst_feats, np.float32)
    reward = np.asarray(reward, np.float32)
    src = np.asarray(src, np.int64)
    dst = np.asarray(dst, np.int64)
    W_ns = np.asarray(W_ns, np.float32)
    b_ns = np.asarray(b_ns, np.float32)
    W_ni = np.asarray(W_ni, np.float32)
    W_nj = np.asarray(W_nj, np.float32)
    W_fij = np.asarray(W_fij, np.float32)
    attn = np.asarray(attn, np.float32)
    b_e = np.asarray(b_e, np.float32)

    # ---- sort by dst and shard at dst boundaries --------------------------
    order = np.argsort(dst, kind="stable")
    d_s = dst[order]
    s_s = src[order]
    r_s = reward[order]

    cut = [0]
    for c in range(1, N_CORES):
        t = (e_tot * c) // N_CORES
        while t < e_tot and t > 0 and d_s[t] == d_s[t - 1]:
            t += 1
        cut.append(t)
    cut.append(e_tot)

    # ---- greedy window packing per core (dense rank-based rows) -----------
    per_core = []
    for c in range(N_CORES):
        e0, e1 = cut[c], cut[c + 1]
        d = d_s[e0:e1]
        wins = []  # (w_start, w_count, n_uniq) over local positions
        if e1 > e0:
            uniq, starts = np.unique(d, return_index=True)
            ends = np.append(starts[1:], len(d))
            w_start = 0
            w_count = 0
            w_uniq = 0
            for gi in range(len(uniq)):
                glen = int(ends[gi] - starts[gi])
                if w_uniq + 1 > 128 or w_count + glen > slots:
                    wins.append((w_start, w_count, w_uniq))
                    w_start = int(starts[gi])
                    w_count = 0
                    w_uniq = 0
                w_count += glen
                w_uniq += 1
            wins.append((w_start, w_count, w_uniq))
        per_core.append((e0, e1, wins))

    W = max(1, max(len(pc[2]) for pc in per_core))

    # virtual feature rows: x_row @ W_nj == colsum(W_fij); y_row @ W_nj == b_e
    wsum = W_fij.sum(axis=0)
    x_row = np.linalg.lstsq(W_nj.T.astype(np.float64), wsum.astype(np.float64),
                            rcond=None)[0].astype(np.float32)
    y_row = np.linalg.lstsq(W_nj.T.astype(np.float64), b_e.astype(np.float64),
                            rcond=None)[0].astype(np.float32)

    F8 = np.dtype("float8_e4m3")
    mf_all = []     # [128, W, t_w, 128] bf16 one-hot (rank) per slot
    zfs_all = []    # [128, W*slots] bf16 src feats (payload path)
    zf8_all = []    # [64, 2, W*2*slots] fp8 src|dst feats (FE DoubleRow path)
    asm = []        # per core (slot_rows, global_rows)

    for c in range(N_CORES):
        e0, e1, wins = per_core[c]
        d = d_s[e0:e1]
        s = s_s[e0:e1]
        r = r_s[e0:e1]

        drel = np.full((W, slots), -1.0, np.float32)
        nfe = np.zeros((W * slots, IN_NODE), np.float32)
        dfe = np.zeros((W * slots, IN_NODE), np.float32)
        rows_slot = []
        rows_glob = []
        for w, (ws, wc, nu) in enumerate(wins):
            sl = slice(ws, ws + wc)
            dsl = d[sl]
            uds, rank = np.unique(dsl, return_inverse=True)
            drel[w, :wc] = rank.astype(np.float32)
            nfe[w * slots:w * slots + wc] = nfeats[s[sl]]
            dfe[w * slots:w * slots + wc] = (dst_feats[dsl]
                                             + r[sl, None] * x_row[None, :]
                                             + y_row[None, :])
            rows_slot.append(w * 128 + np.arange(len(uds)))
            rows_glob.append(uds)

        # one-hot per slot, layout [128 p, W, t, 128 dcol]
        ohm = (drel.reshape(W, t_w, 128)[:, :, :, None]
               == np.arange(128, dtype=np.float32)).astype(BF16)
        ohm = np.ascontiguousarray(ohm.transpose(2, 0, 1, 3))

        zfs = np.ascontiguousarray(
            nfe.T.reshape(IN_NODE, W * slots).astype(BF16))
        # fp8 DoubleRow pack [128, 2, W*slots]: j=0 src feats, j=1 dst feats
        # (one DR matmul contracts over 256 = 128 src-feat + 128 dst-feat)
        zf8 = np.empty((IN_NODE, 2, W * slots), np.float32)
        zf8[:, 0, :] = nfe.T.reshape(IN_NODE, W * slots)
        zf8[:, 1, :] = dfe.T.reshape(IN_NODE, W * slots)
        mf_all.append(ohm.astype(F8))
        zfs_all.append(zfs)
        zf8_all.append(np.ascontiguousarray(zf8.astype(F8)))
        asm.append((np.concatenate(rows_slot) if rows_slot else
                    np.zeros(0, np.int64),
                    np.concatenate(rows_glob) if rows_glob else
                    np.zeros(0, np.int64)))

    # ---- shared constants -------------------------------------------------
    # payload columns f-major: col f*H + h  <-  W_ns[:, h*OUT_NODE + f]
    wpay = (W_ns * 0.25).reshape(IN_NODE, H, OUT_NODE)   # fold mean over heads
    wpay = np.ascontiguousarray(wpay.transpose(0, 2, 1)).reshape(IN_NODE, NPAY)
    wpay = wpay.astype(BF16)                                       # [128,256]
    # FE weights fp8 DoubleRow-packed [128, 2, FE]: j=0 W_ni, j=1 W_nj
    wfej8 = np.empty((IN_NODE, 2, FE), np.float32)
    wfej8[:, 0, :] = W_ni
    wfej8[:, 1, :] = W_nj
    wfej8 = np.ascontiguousarray(wfej8.astype(F8))
    # block-diag attention for the PE dot: attnblk[h*16+f, h'] = attn[h,f]
    # if h'==h else 0
    blk = np.zeros((FE, H), np.float32)
    for h in range(H):
        blk[h * OUT_EDGE:(h + 1) * OUT_EDGE, h] = attn[h]
    attnblk = np.ascontiguousarray(
        np.concatenate([blk, blk], axis=0).astype(BF16))   # [128, 4]
    bmean = np.broadcast_to(b_ns.reshape(H, OUT_NODE).mean(axis=0),
                            (128, OUT_NODE)).astype(np.float32).copy()

    in_maps = []
    for c in range(N_CORES):
        in_maps.append(dict(
            zfs=zfs_all[c], zf8=zf8_all[c], ohm=mf_all[c],
            wfej8=wfej8, wpay=wpay, attnblk=attnblk, bmean=bmean,
        ))

    meta = dict(W=W, asm=asm, cfg=cfg)
    return meta, in_maps


# ===========================================================================
# Device program
# ===========================================================================

def build_program(W, cfg):
    import concourse.bacc as bacc
    import concourse.tile as tile
    import concourse.mybir as mybir
    from contextlib import ExitStack

    dt = mybir.dt
    AF = mybir.ActivationFunctionType
    OP = mybir.AluOpType

    t_half = cfg["t_half"]
    t_w = 2 * t_half
    slots = t_w * 128

    nc = bacc.Bacc(None, target_bir_lowering=False)

    ZFS = nc.declare_dram_parameter("zfs", [IN_NODE, W * slots],
                                    dt.bfloat16, isOutput=False)
    ZF8 = nc.declare_dram_parameter("zf8", [IN_NODE, 2, W * slots],
                                    dt.float8e4, isOutput=False)
    OHM = nc.declare_dram_parameter("ohm", [128, W, t_w, 128], dt.float8e4,
                                    isOutput=False)
    WFEJ8 = nc.declare_dram_parameter("wfej8", [IN_NODE, 2, FE], dt.float8e4,
                                      isOutput=False)
    WPAY = nc.declare_dram_parameter("wpay", [IN_NODE, NPAY], dt.bfloat16,
                                     isOutput=False)
    ATTN = nc.declare_dram_parameter("attnblk", [128, H], dt.bfloat16,
                                     isOutput=False)
    BMEAN = nc.declare_dram_parameter("bmean", [128, OUT_NODE], dt.float32,
                                      isOutput=False)
    OUT = nc.declare_dram_parameter("out", [W * 128, OUT_NODE], dt.float32,
                                    isOutput=True)

    with tile.TileContext(nc) as tc, ExitStack() as ctx:
        cpool = ctx.enter_context(tc.tile_pool(name="consts", bufs=1))
        wfe_s = cpool.tile([128, 2, FE], dt.float8e4)
        nc.sync.dma_start(wfe_s[:], WFEJ8[:])
        wpay_s = cpool.tile([128, NPAY], dt.bfloat16)
        nc.sync.dma_start(wpay_s[:], WPAY[:])
        attn_s = cpool.tile([128, H], dt.bfloat16)
        nc.sync.dma_start(attn_s[:], ATTN[:])
        bmean_s = cpool.tile([128, OUT_NODE], dt.float32)
        nc.sync.dma_start(bmean_s[:], BMEAN[:])

        with tc.tile_pool(name="feat", bufs=4) as fpool, \
             tc.tile_pool(name="meta", bufs=4) as mpool, \
             tc.tile_pool(name="work", bufs=4) as wpool, \
             tc.tile_pool(name="stg", bufs=4) as spool, \
             tc.tile_pool(name="rhsp", bufs=4) as rpool, \
             tc.tile_pool(name="ep", bufs=3) as epool, \
             tc.tile_pool(name="psFE", bufs=1, space="PSUM") as psFE, \
             tc.tile_pool(name="psPay", bufs=3, space="PSUM") as psPay, \
             tc.tile_pool(name="psE", bufs=1, space="PSUM") as psE, \
             tc.tile_pool(name="psP", bufs=2, space="PSUM") as psP:
            # the scatter + epilogue of window w are emitted at the start of
            # window w+1 (software pipelining): the previous window's scatter
            # fills the PE while Act runs this window's leaky_relu.
            pending = None      # (w, ohw, rhs, P)

            def flush(pend):
                pw, p_ohw, p_rhs, p_P = pend
                # ---- one-hot scatter matmul -------------------------
                for t in range(t_w):
                    nc.tensor.matmul(p_P[:], lhsT=p_ohw[:, t, :],
                                     rhs=p_rhs[:, t, :],
                                     start=(t == 0), stop=(t == t_w - 1),
                                     skip_group_check=True)
                # ---- epilogue ---------------------------------------
                si = epool.tile([128, H], dt.float32, tag="si")
                nc.vector.reciprocal(si[:], p_P[:, NPAY:])
                tmp = epool.tile([128, OUT_NODE, H], dt.float32, tag="tmp")
                nc.vector.tensor_tensor(
                    out=tmp[:],
                    in0=p_P[:, 0:NPAY].rearrange("p (f h) -> p f h", h=H),
                    in1=si[:].unsqueeze(1).broadcast_to([128, OUT_NODE, H]),
                    op=OP.mult)
                acc = epool.tile([128, OUT_NODE], dt.float32, tag="acc")
                nc.vector.tensor_reduce(acc[:], tmp[:],
                                        axis=mybir.AxisListType.X, op=OP.add)
                acc2 = epool.tile([128, OUT_NODE], dt.float32, tag="acc2")
                nc.gpsimd.tensor_add(acc2[:], acc[:], bmean_s[:])
                if pw % 4 == 0:
                    res_l[0] = epool.tile([128, 4, OUT_NODE], dt.float32,
                                          tag="res", name=f"res{pw}")
                nc.gpsimd.tensor_scalar(out=res_l[0][:, pw % 4, :],
                                        in0=acc2[:], scalar1=0.0,
                                        scalar2=None, op0=OP.max)
                if pw % 4 == 3 or pw == W - 1:
                    w0 = (pw // 4) * 4
                    nc.sync.dma_start(
                        OUT[w0 * 128:(pw + 1) * 128, :]
                        .rearrange("(b p) c -> p b c", p=128),
                        res_l[0][:, 0:pw - w0 + 1, :])

            res_l = [None]
            for w in range(W):
                zfs = fpool.tile([128, slots], dt.bfloat16, tag="zfs")
                nc.sync.dma_start(zfs[:], ZFS[:, w * slots:(w + 1) * slots])
                zf8 = fpool.tile([128, 2, slots], dt.float8e4, tag="zf8")
                nc.sync.dma_start(zf8[:],
                                  ZF8[:, :, w * slots:(w + 1) * slots])
                ohw = mpool.tile([128, t_w, 128], dt.float8e4, tag="ohw")
                nc.gpsimd.dma_start(ohw[:], OHM[:, w, :, :])

                rhs = rpool.tile([128, t_w, NTOT], dt.bfloat16, tag="rhs")
                eatp = psE.tile([128, t_w, H], dt.float32, tag="eatp")
                P = psP.tile([128, NTOT], dt.float32, tag="P")

                for hf in range(2):
                    ts0 = hf * t_half
                    hb = ts0 * 128          # first slot of the half
                    # ---- FE projection, FE-major: out[fe, slot] for the
                    # 1024 slots of the half, written at partition base 0
                    # (HW requires DoubleRow output at partition 0).  One DR
                    # matmul contracts 128 src + 128 dst features.
                    prFE = psFE.tile([64, t_half * 128], dt.float32,
                                     tag="prFE")
                    for b in range(2):      # matmul out must fit one bank
                        s0 = hb + b * 512
                        nc.tensor.matmul(prFE[:, b * 512:(b + 1) * 512],
                                         lhsT=wfe_s[:],
                                         rhs=zf8[:, :, s0:s0 + 512],
                                         perf_mode=mybir.MatmulPerfMode
                                         .DoubleRow,
                                         start=True, stop=True,
                                         skip_group_check=True)
                    # ---- leaky relu on Act (Prelu alpha=slope) -----------
                    lr = wpool.tile([64, t_half * 128], dt.bfloat16,
                                    tag="lr")
                    nc.scalar.activation(lr[:], prFE[:], AF.Prelu, alpha=SLOPE)
                    # previous window's scatter+epilogue go here: they keep
                    # the PE busy while Act runs this half's leaky
                    if hf == 0 and pending is not None:
                        flush(pending)
                        pending = None
                    # ---- attention dot on PE: e[slot,h] = lr_fe . attnblk
                    for t in range(t_half):
                        c0 = t * 128
                        nc.tensor.matmul(
                            eatp[:, ts0 + t, :],
                            lhsT=lr[:, c0:c0 + 128],
                            rhs=attn_s[0:64, :],
                            start=True, stop=True, skip_group_check=True)
                    # ---- exp -> wgt columns of rhs -----------------------
                    nc.scalar.activation(rhs[:, ts0:ts0 + t_half, NPAY:],
                                         eatp[:, ts0:ts0 + t_half, :], AF.Exp)

                    # ---- payload projection + weight-mult, 2-tile chunks -
                    # direct chunks go straight through DVE (PSUM read);
                    # staged chunks are copied to SBUF by Act then weighted
                    # on Pool (GPSIMD cannot touch PSUM).
                    for cch in range(t_half // 2):
                        tb = ts0 + cch * 2
                        prP = psPay.tile([128, 2, NPAY], dt.float32, tag="prP")
                        for k in range(2):
                            c0 = (tb + k) * 128
                            nc.tensor.matmul(prP[:, k, :],
                                             lhsT=zfs[:, c0:c0 + 128],
                                             rhs=wpay_s[:], start=True,
                                             stop=True, skip_group_check=True)
                        def wmul_direct(lo, hi):
                            nc.vector.tensor_tensor(
                                out=rhs[:, tb + lo:tb + hi, 0:NPAY]
                                .rearrange("p t (f h) -> p t f h", h=H),
                                in0=prP[:, lo:hi, :]
                                .rearrange("p t (f h) -> p t f h", h=H),
                                in1=rhs[:, tb + lo:tb + hi, NPAY:]
                                .unsqueeze(2)
                                .broadcast_to([128, hi - lo, OUT_NODE, H]),
                                op=OP.mult)

                        def wmul_staged(lo, hi):
                            stg = spool.tile([128, 2, NPAY], dt.bfloat16,
                                             tag="stg")
                            nc.scalar.copy(stg[:, lo:hi, :], prP[:, lo:hi, :])
                            nc.gpsimd.tensor_tensor(
                                out=rhs[:, tb + lo:tb + hi, 0:NPAY]
                                .rearrange("p t (f h) -> p t f h", h=H),
                                in0=stg[:, lo:hi, :]
                                .rearrange("p t (f h) -> p t f h", h=H),
                                in1=rhs[:, tb + lo:tb + hi, NPAY:]
                                .unsqueeze(2)
                                .broadcast_to([128, hi - lo, OUT_NODE, H]),
                                op=OP.mult)

                        # 3 chunks staged (Act -> Pool), 5 direct on DVE;
                        # later chunks direct (short path so the scatter
                        # isn't stalled behind the Act -> Pool chain)
                        if cch < (2 if hf == 0 else 1):
                            wmul_staged(0, 2)
                        else:
                            wmul_direct(0, 2)

                pending = (w, ohw, rhs, P)
            flush(pending)

    if not nc.is_finalized():
        nc.finalize()
    return nc


# ===========================================================================
# numpy emulation of the device program (for validation/debug)
# ===========================================================================

def emulate_core(in_map, W, cfg):
    t_half = cfg["t_half"]
    slots = 2 * t_half * 128

    f32 = np.float32
    wfe = in_map["wfej8"][:, 0, :].astype(f32)
    wnj = in_map["wfej8"][:, 1, :].astype(f32)
    wpay = in_map["wpay"].astype(f32)
    attnblk = in_map["attnblk"][0:FE, :].astype(f32)     # [64, 4]
    bmean = in_map["bmean"][0]

    out = np.zeros((W * 128, OUT_NODE), f32)
    for w in range(W):
        zf8 = in_map["zf8"][:, :, w * slots:(w + 1) * slots].astype(f32)
        nf8 = zf8[:, 0, :].T
        df8 = zf8[:, 1, :].T
        nfe = in_map["zfs"][:, w * slots:(w + 1) * slots].astype(f32).T
        fout = (nf8 @ wfe + df8 @ wnj)                   # [slots, 64] psum f32
        lr = np.where(fout > 0, fout, SLOPE * fout).astype(BF16).astype(f32)
        eat = lr @ attnblk                               # PE dot, psum f32
        wgt = np.exp(eat).astype(BF16).astype(f32)       # [slots, H]
        pay = (nfe @ wpay)                               # [slots, 256] f-major
        rhs_pay = (pay.reshape(-1, OUT_NODE, H)
                   * wgt[:, None, :]).reshape(-1, NPAY).astype(BF16).astype(f32)
        oh = (in_map["ohm"][:, w].astype(f32).transpose(1, 0, 2)
              .reshape(slots, 128))
        P = oh.T @ np.concatenate([rhs_pay, wgt], axis=1)   # [128, 260]
        s = np.maximum(P[:, NPAY:], 1e-30)
        si = 1.0 / s
        acc = (P[:, 0:NPAY].reshape(128, OUT_NODE, H) * si[:, None, :]
               ).sum(axis=2)
        out[w * 128:(w + 1) * 128] = np.maximum(acc + bmean[None, :], 0)
    return out


def assemble(meta, results):
    n_dst = meta["cfg"]["n_dst"]
    out = np.zeros((n_dst, OUT_NODE), np.float32)
    for c in range(N_CORES):
        slots_rows, glob_rows = meta["asm"][c]
        if len(glob_rows):
            out[glob_rows] = results[c]["out"][slots_rows]
    return out


# ===========================================================================
# entry point
# ===========================================================================

_CACHE = {}
LAST_EXEC_NS = None
LAST_RESULT = None


def kernel(nfeats, dst_feats, reward, src, dst,
           W_ns, b_ns, W_ni, W_nj, W_fij, attn, b_e):
    global LAST_EXEC_NS, LAST_RESULT
    import os
    from concourse.bass_utils import run_bass_kernel_spmd

    meta, in_maps = prep(nfeats, dst_feats, reward, src, dst,
                         W_ns, b_ns, W_ni, W_nj, W_fij, attn, b_e)
    key = meta["W"]
    if key not in _CACHE:
        _CACHE[key] = build_program(meta["W"], meta["cfg"])
    nc = _CACHE[key]
    kwargs = {}
    if os.environ.get("EGAT_TRACE"):
        kwargs = dict(trace=True)
    try:
        res = run_bass_kernel_spmd(nc, in_maps, list(range(N_CORES)), **kwargs)
    except ModuleNotFoundError:
        # NTFF profile hook unavailable in this environment
        res = run_bass_kernel_spmd(nc, in_maps, list(range(N_CORES)))
    LAST_EXEC_NS = res.exec_time_ns
    LAST_RESULT = res
    return assemble(meta, res.results)


def estimate_ns(W=None, cfg=None):
    """Cost-model (no_exec CoreSim) estimate of the per-core kernel time."""
    from concourse.bass_interp import CoreSim
    cfg = cfg or default_cfg()
    if W is None:
        W = sorted(_CACHE)[0] if _CACHE else 50
    nc = _CACHE.get(W) or build_program(W, cfg)
    sim = CoreSim(nc, no_exec=True)
    sim.simulate()
    return int(sim.time)
